# revision 42
# baseline (speedup 1.0000x reference)
"""Trainium2 Bass kernel for nn_MHA_2516850835986.

MHA: B=1, T=2048, C=2048, H=32 heads, d=64, causal, RoPE (head-indexed
angle quirk: within head h all feature pairs rotate by t * 10000^(-h/32)).

Sharding: head-parallel across 8 cores (4 heads each). x is replicated
(pre-transposed on host), qkv columns / proj rows sharded by head. Each
core produces a partial [T, C] output (proj contraction over its own
heads' features); partials are summed on host.

v6 design (v5 ~244us, v4 246.7us, v3 299us, f32r baseline 425us;
v6 best sample 236.8us, band ~237-246 dominated by cross-core HBM
phase variance):
- Preamble DMA in exact consumption order: wqk|xt quarters, then wv
  halves split across both queues (the v chains consume all 16 kc right
  after qk), then cos/sin. NWARM=24 bridges the warm-up to the first
  DMA-gated matmul so HAM opens at ~11us and stays open ~185us.
- passB rotates 4 psum chains (2-bank sps slab + two unips banks) so
  its evac copies pipeline fully behind the matmuls; junk keep-warm
  tile rides yps (free at the tail).

v5 design notes:
- ACT kept pure-exp through attention: all other psum evacuations ride
  DVE (GpSimd has no PSUM port); wide [128,1024] psum slabs evac'd with
  ACT+DVE halves in parallel during the exp-free proj windows.
- Per-bp inline softmax tails (no deferred pile-up at tile boundaries).
- Tile-3's v-GEMM + tile-2's proj + tile-3's b=0 proj half all weave
  into tile-3's exp-bound attention; the b=0 partial ships to a second
  DRAM output (out2) summed by the host like any core partial, so only
  the b=1 half + out-DMA remain after the last attention step.
- DMA: both HW queues carry the tile-0 critical stream in consumption
  order (wqk|xt quarters, then cos/sin, then wv split across queues);
  out-DMAs alternate queues; SWDGE only for the tiny masks (its early
  issue steals HBM bandwidth, and it is too slow for strided tables).
- Junk keep-warm matmuls thread the tile-3 tail so the HAM clock gate
  stays at 2.4 GHz through passB.

v4 design notes:
- bf16 matmul streams everywhere (tol 2e-2; measured v3 err 5.5e-3).
- Software-pipelined EMISSION: the PE executes in strict pc order, so
  tile i+1's qk/v GEMM is emitted in ~4-matmul chunks BETWEEN the
  scores->av steps of tile i's attention. The ~870ns exp latency per
  step is hidden behind next-tile GEMM work instead of stalling the PE.
- Tile-0 qk runs kc-outer (4 concurrent psum chains) so each arriving
  1MB DMA quarter immediately unlocks 16 matmuls: the preamble streams.
- Diagonal score blocks narrowed to causal width; only the 128-wide
  corner is masked (gpsimd, bf16).
- RoPE fused into the qk-psum evacuation (qcos/qsin), sign folded into
  sintab, swap via PE perm matmul.
- reciprocal_approx_fast on a partition-0 staged denominator row (the
  custom DVE op mishandles partition-shifted APs - learned the NaN way).
- PE warm-up burst at t=0 keeps the HAM clock gate at 2.4 GHz.
"""

import sys

sys.path.insert(0, "/opt/trn_rl_repo")
import numpy as np

T = 2048
C = 2048
NH = 32          # total heads
HL = 4           # heads per core
D = 64           # head dim
NC_ = 8          # cores
TT = 512         # t-tile width
NTT = T // TT    # 4 t-tiles
KC = C // 128    # 16 contraction chunks
ROPE_THETA = 10000.0

_CACHE = {}


def _build_program():
    import concourse.bass as bass
    import concourse.tile as tile
    from concourse import bacc, mybir
    from contextlib import ExitStack

    F32 = mybir.dt.float32
    F32R = mybir.dt.float32r
    BF16 = mybir.dt.bfloat16
    EXP = mybir.ActivationFunctionType.Exp
    MUL = mybir.AluOpType.mult
    ADD = mybir.AluOpType.add

    nc = bacc.Bacc(None, target_bir_lowering=False)

    xt = nc.declare_dram_parameter("xt", [C, T], BF16, False)          # x^T
    wqk = nc.declare_dram_parameter("wqk", [C, 4 * 128], BF16, False)  # q|k cols
    wv = nc.declare_dram_parameter("wv", [C, 256], BF16, False)
    wproj = nc.declare_dram_parameter("wproj", [256, T], BF16, False)
    costab = nc.declare_dram_parameter("costab", [128, 2, T], BF16, False)
    sintab = nc.declare_dram_parameter("sintab", [128, 2, T], BF16, False)
    tri = nc.declare_dram_parameter("tri", [128, 128], BF16, False)    # corner keep-mask
    perm = nc.declare_dram_parameter("perm", [128, 128], BF16, False)  # pair-swap
    out = nc.declare_dram_parameter("out", [T, T], BF16, True)
    # tile-3 proj b=0 partial (computed early, woven into tile-3's
    # attention; host adds it into rows 1536:2048 like any other partial)
    out2 = nc.declare_dram_parameter("out2", [TT, T], BF16, True)

    xt_v = xt.rearrange("(kc p) t -> p kc t", p=128)
    wqk_v = wqk.rearrange("(kc p) m -> p kc m", p=128)
    wv_v = wv.rearrange("(kc p) m -> p kc m", p=128)
    wproj_v = wproj.rearrange("(b p) n -> p b n", p=128)

    with tile.TileContext(nc) as tc, ExitStack() as ctx:
        consts = ctx.enter_context(tc.tile_pool(name="consts", bufs=1))
        xtp = ctx.enter_context(tc.tile_pool(name="xtp", bufs=6))
        csp = ctx.enter_context(tc.tile_pool(name="csp", bufs=4))
        ropep = ctx.enter_context(tc.tile_pool(name="ropep", bufs=2))
        qrotp = ctx.enter_context(tc.tile_pool(name="qrotp", bufs=2))
        persist = ctx.enter_context(tc.tile_pool(name="persist", bufs=1))
        p4p = ctx.enter_context(tc.tile_pool(name="p4p", bufs=2))
        ytp = ctx.enter_context(tc.tile_pool(name="ytp", bufs=2))
        ytmpp = ctx.enter_context(tc.tile_pool(name="ytmpp", bufs=2))
        ymp = ctx.enter_context(tc.tile_pool(name="ymp", bufs=4))
        rp = ctx.enter_context(tc.tile_pool(name="rp", bufs=4))
        outp = ctx.enter_context(tc.tile_pool(name="outp", bufs=4))

        # PSUM: S2 pairs (2 banks x2) + y (1 bank x2) + everything else (1 bank x2)
        sps = ctx.enter_context(tc.tile_pool(name="sps", bufs=2, space="PSUM"))
        yps = ctx.enter_context(tc.tile_pool(name="yps", bufs=2, space="PSUM"))
        unips = ctx.enter_context(tc.tile_pool(name="unips", bufs=2, space="PSUM"))

        wqk_sb = consts.tile([128, KC, 512], BF16)
        wv_sb = consts.tile([128, KC, 256], BF16)
        wproj_sb = consts.tile([128, 2, T], BF16)
        tri_sb = consts.tile([128, 128], BF16)
        perm_sb = consts.tile([128, 128], BF16)
        ones_sb = consts.tile([128, 64], F32R)
        nc.vector.memset(ones_sb[:].bitcast(F32), 1.0)

        # ---- PE warm-up: junk matmuls so the HAM activity window sees a
        # busy PE during the DMA preamble and the clock gate opens to
        # 2.4 GHz before the first real matmul ----
        warm_sb = consts.tile([128, 256], BF16)
        junk_sb = consts.tile([1, 8], F32)
        nc.gpsimd.memset(warm_sb[:], 0.25)
        wps = unips.tile([128, TT], F32, tag="uni")
        NWARM = 24
        for w in range(NWARM):
            nc.tensor.matmul(wps[:, 0:256], warm_sb[:, 0:128], warm_sb[:],
                             start=(w == 0), stop=(w == NWARM - 1))
        nc.vector.tensor_copy(junk_sb[:], wps[0:1, 0:8])  # keep-alive consumer

        # v in normal layout [s, dd]: per s-block slot of 4 heads x (64 v + 1 one + 1 pad)
        v_sb = persist.tile([128, KC, HL, 66], BF16)
        nc.vector.memset(v_sb[:].rearrange("p a b c -> p (a b c)"), 1.0)
        # k^T (rope'd), persistent across tiles: [dd(2 heads), block, t]
        krot = persist.tile([128, 2, T], BF16)

        def load_tile(j):
            """Issue input DMAs for t-tile j (xt halves split across the
            two HW queues, cos/sin right behind them)."""
            tslj = slice(TT * j, TT * (j + 1))
            xth = []
            for half in range(2):
                xh = xtp.tile([128, KC // 2, TT], BF16, tag="xt")
                eng = nc.sync if half == 0 else nc.scalar
                eng.dma_start(xh[:], xt_v[:, (KC // 2) * half:(KC // 2) * (half + 1), tslj])
                xth.append(xh)
            cos_t = csp.tile([128, 2, TT], BF16, tag="cos")
            nc.sync.dma_start(cos_t[:], costab[:, :, tslj])
            sin_t = csp.tile([128, 2, TT], BF16, tag="sin")
            nc.scalar.dma_start(sin_t[:], sintab[:, :, tslj])
            return xth, cos_t, sin_t

        # ---- preamble: tile-0 inputs interleaved with wqk in quarter
        # chunks across both HW queues, in exact consumption order; only
        # the tiny masks ride the gpsimd SWDGE queue (big transfers there
        # would steal HBM bandwidth from the critical stream since SWDGE
        # issues immediately) ----
        xh0 = xtp.tile([128, KC // 2, TT], BF16, tag="xt")
        xh1 = xtp.tile([128, KC // 2, TT], BF16, tag="xt")
        xq = [xh0[:, 0:4, :], xh0[:, 4:8, :], xh1[:, 0:4, :], xh1[:, 4:8, :]]
        nc.gpsimd.dma_start(perm_sb[:], perm[:])
        nc.gpsimd.dma_start(tri_sb[:], tri[:])
        for q in range(4):
            nc.sync.dma_start(wqk_sb[:, 4 * q:4 * (q + 1), :],
                              wqk_v[:, 4 * q:4 * (q + 1), :])
            nc.scalar.dma_start(xq[q], xt_v[:, 4 * q:4 * (q + 1), 0:TT])
        # wv halves FIRST (the v chains consume all 16 kc chunks right
        # after the qk chains, ~2us before the rope needs cos/sin)
        nc.sync.dma_start(wv_sb[:, 0:8, :], wv_v[:, 0:8, :])
        nc.scalar.dma_start(wv_sb[:, 8:16, :], wv_v[:, 8:16, :])
        cos0 = csp.tile([128, 2, TT], BF16, tag="cos")
        nc.sync.dma_start(cos0[:], costab[:, :, 0:TT])
        sin0 = csp.tile([128, 2, TT], BF16, tag="sin")
        nc.scalar.dma_start(sin0[:], sintab[:, :, 0:TT])
        loads = [([xh0, xh1], cos0, sin0)]
        # prefetch tile 1 behind the tile-0 critical stream; wproj last
        # (first consumer is tile-0's proj at ~60us)
        loads.append(load_tile(1))

        qrots = {}
        yts = {}

        def emit_rope(m, ps, cos_t, sin_t, qrot, i):
            """Fused RoPE evacuation of one qk psum chain."""
            bb = m % 2
            qcos = ropep.tile([128, TT], BF16, tag="qcos")
            nc.vector.tensor_tensor(qcos[:], ps[:], cos_t[:, bb, :], MUL)
            qsin = ropep.tile([128, TT], BF16, tag="qsin")
            nc.vector.tensor_tensor(qsin[:], ps[:], sin_t[:, bb, :], MUL)
            psw = unips.tile([128, TT], F32, tag="uni")
            nc.tensor.matmul(psw[:], perm_sb[:], qsin[:], start=True, stop=True)
            dst = qrot[:, bb, :] if m < 2 else krot[:, bb, TT * i:TT * (i + 1)]
            nc.vector.tensor_tensor(dst, qcos[:], psw[:], ADD)

        def gemm_chunks(i):
            """Build tile i's qk+v GEMM as two lists of closures (qk+rope,
            then v), each chunk emitting ~4 matmuls, to be woven between
            attention steps."""
            xth, cos_t, sin_t = loads[i]
            qrot = qrotp.tile([128, 2, TT], BF16, tag="qrot")
            qrots[i] = qrot
            chunks = []
            for m in range(4):
                cell = {}

                def qk_chunk(m=m, q4=0, cell=cell):
                    if q4 == 0:
                        cell["ps"] = unips.tile([128, TT], F32, tag="uni", name="ps")
                    ps = cell["ps"]
                    for kc in range(4 * q4, 4 * q4 + 4):
                        nc.tensor.matmul(ps[:], wqk_sb[:, kc, 128 * m:128 * (m + 1)],
                                         xth[kc // 8][:, kc % 8, :],
                                         start=(kc == 0), stop=(kc == KC - 1))
                    if q4 == 3:
                        emit_rope(m, ps, cos_t, sin_t, qrot, i)

                for q4 in range(4):
                    chunks.append(lambda m=m, q4=q4, cell=cell: qk_chunk(m, q4, cell))
            vchunks = []
            for tc4 in range(4):
                cell = {}

                def v_chunk(tc4=tc4, q4=0, cell=cell):
                    if q4 == 0:
                        cell["ps"] = unips.tile([128, TT], F32, tag="uni", name="psv")
                    psv = cell["ps"]
                    for kc in range(4 * q4, 4 * q4 + 4):
                        nc.tensor.matmul(psv[:, 0:256],
                                         xth[kc // 8][:, kc % 8, 128 * tc4:128 * (tc4 + 1)],
                                         wv_sb[:, kc, :],
                                         start=(kc == 0), stop=(kc == KC - 1))
                    if q4 == 3:
                        nc.vector.tensor_copy(
                            v_sb[:, 4 * i + tc4, :, 0:64],
                            psv[:, 0:256].rearrange("p (h d) -> p h d", h=HL))

                for q4 in range(4):
                    vchunks.append(lambda tc4=tc4, q4=q4, cell=cell: v_chunk(tc4, q4, cell))
            return chunks, vchunks

        # ---- tile 0 GEMM inline, kc-outer so each arriving DMA quarter
        # (wqk q + xt q) unlocks 16 matmuls across 4 concurrent chains ----
        xth0, cos_t0, sin_t0 = loads[0]
        qrot0 = qrotp.tile([128, 2, TT], BF16, tag="qrot")
        qrots[0] = qrot0
        ps_m = [unips.tile([128, TT], F32, tag="uni", name="ps_m0"),
                unips.tile([128, TT], F32, tag="uni", name="ps_m1"),
                yps.tile([128, TT], F32, tag="y", name="ps_m2"),
                yps.tile([128, TT], F32, tag="y", name="ps_m3")]
        jps = sps.tile([128, 2 * TT], F32, tag="S", name="jps")
        nj = 0
        for kc in range(KC):
            for m in range(4):
                nc.tensor.matmul(ps_m[m][:], wqk_sb[:, kc, 128 * m:128 * (m + 1)],
                                 xth0[kc // 8][:, kc % 8, :],
                                 start=(kc == 0), stop=(kc == KC - 1))
            if kc % 4 == 3 and kc < KC - 1:
                for w in range(10):
                    nc.tensor.matmul(jps[:, 0:256], warm_sb[:, 0:128], warm_sb[:],
                                     start=(nj == 0), stop=(nj == 29))
                    nj += 1
        nc.vector.tensor_copy(junk_sb[:], jps[0:1, 0:8])  # release the S slot
        # v chain first, then rope: the PE streams the v GEMM while the DVE
        # works through the rope's elementwise ops
        for m in range(4):
            psv = unips.tile([128, TT], F32, tag="uni", name="psv0")
            for kc in range(KC):
                nc.tensor.matmul(psv[:, 0:256],
                                 xth0[kc // 8][:, kc % 8, 128 * m:128 * (m + 1)],
                                 wv_sb[:, kc, :],
                                 start=(kc == 0), stop=(kc == KC - 1))
            emit_rope(m, ps_m[m], cos_t0, sin_t0, qrot0, 0)
            nc.vector.tensor_copy(
                v_sb[:, m, :, 0:64],
                psv[:, 0:256].rearrange("p (h d) -> p h d", h=HL))

        def proj_block(j, ytj, tc4, ct, cell, pool, dve_only=False):
            """One [128,512] slab of tile j's proj: 2 matmuls + copy (+DMA)."""
            if ct == 0:
                cell["osb"] = outp.tile([128, 4 * TT], BF16, tag="osb", name="osb")
            osb = cell["osb"]
            pso = pool.tile([128, TT], F32, tag=("uni" if pool is unips else "y"),
                            name="pso")
            for b in range(2):
                nc.tensor.matmul(pso[:],
                                 ytj[:, b, 128 * tc4:128 * (tc4 + 1)],
                                 wproj_sb[:, b, TT * ct:TT * (ct + 1)],
                                 start=(b == 0), stop=(b == 1))
            if ct % 2 == 0 and not dve_only:
                nc.scalar.copy(osb[:, TT * ct:TT * (ct + 1)], pso[:])
            else:
                nc.vector.tensor_copy(osb[:, TT * ct:TT * (ct + 1)], pso[:])
            if ct == 3:
                nc.sync.dma_start(
                    out[TT * j + 128 * tc4: TT * j + 128 * (tc4 + 1), :],
                    osb[:])

        def emit_proj(j, ytj):
            """Partial out rows for t-tile j: [128,1024] psum slabs (wide
            bf16 moving operand), one wide evac copy per slab, out-DMAs
            balanced across both HW queues."""
            for tc4 in range(4):
                osb = outp.tile([128, 4 * TT], BF16, tag="osb", name="osb")
                for cp in range(2):
                    pso2 = sps.tile([128, 2 * TT], F32, tag="S", name="pso2")
                    for h in range(2):
                        ct = 2 * cp + h
                        for b in range(2):
                            nc.tensor.matmul(pso2[:, TT * h:TT * (h + 1)],
                                             ytj[:, b, 128 * tc4:128 * (tc4 + 1)],
                                             wproj_sb[:, b, TT * ct:TT * (ct + 1)],
                                             start=(b == 0), stop=(b == 1))
                    # split the wide evac across ACT+DVE so they run
                    # concurrently (no exp in this window)
                    nc.scalar.copy(osb[:, 2 * TT * cp:2 * TT * cp + TT],
                                   pso2[:, 0:TT])
                    nc.vector.tensor_copy(osb[:, 2 * TT * cp + TT:2 * TT * (cp + 1)],
                                          pso2[:, TT:2 * TT])
                eng = nc.sync if tc4 % 2 == 0 else nc.scalar
                eng.dma_start(
                    out[TT * j + 128 * tc4: TT * j + 128 * (tc4 + 1), :],
                    osb[:])

        def proj_chunks(j, ytj):
            """Tile j's proj as weave chunks (pso from the then-idle unips
            pool; copies DVE-only so the weave never steals ACT from the
            host attention's exp stream)."""
            chunks = []
            for tc4 in range(4):
                cell = {}
                for ct in range(4):
                    chunks.append(
                        lambda tc4=tc4, ct=ct, cell=cell:
                            proj_block(j, ytj, tc4, ct, cell, unips, True))
            return chunks

        def passA_chunks(ytj):
            """Tile-3 proj, b=0 contraction half only, as weave chunks.
            Each slab is 1 matmul + a DVE copy into an out2 staging tile;
            the host sums out2 into the final rows like any core partial."""
            chunks = []
            for tc4 in range(4):
                cell = {}

                def f(tc4=tc4, ct=0, cell=cell):
                    if ct == 0:
                        cell["osb"] = outp.tile([128, 4 * TT], BF16, tag="osb",
                                                name="osbA")
                    osb = cell["osb"]
                    pso = unips.tile([128, TT], F32, tag="uni", name="psoA")
                    nc.tensor.matmul(pso[:],
                                     ytj[:, 0, 128 * tc4:128 * (tc4 + 1)],
                                     wproj_sb[:, 0, TT * ct:TT * (ct + 1)],
                                     start=True, stop=True)
                    # mostly DVE; one in four on ACT (its exp stream has a
                    # little slack in the tile-3 bp=1 window, DVE has less)
                    if ct == 1:
                        nc.scalar.copy(osb[:, TT * ct:TT * (ct + 1)], pso[:])
                    else:
                        nc.vector.tensor_copy(osb[:, TT * ct:TT * (ct + 1)], pso[:])
                    if ct == 3:
                        nc.scalar.dma_start(
                            out2[128 * tc4:128 * (tc4 + 1), :], osb[:])

                for ct in range(4):
                    chunks.append(lambda tc4=tc4, ct=ct, cell=cell: f(tc4, ct, cell))
            return chunks

        def emit_passB(ytj, warm):
            """Tile-3 proj b=1 half: paired [128,1024] psum slabs from the
            (now idle) sps pool, evacs split across ACT+DVE, junk matmuls
            interleaved to hold the HAM clock gate open through the tail."""
            for tc4 in range(4):
                osb = outp.tile([128, 4 * TT], BF16, tag="osb", name="osbB")
                # cp=0 slab on a wide sps tile, cp=1 as two narrow unips
                # tiles: 4 psum chains rotate so the evac copies pipeline
                # fully behind the matmuls instead of gating them
                pso2 = sps.tile([128, 2 * TT], F32, tag="S", name="psoB")
                for h in range(2):
                    nc.tensor.matmul(pso2[:, TT * h:TT * (h + 1)],
                                     ytj[:, 1, 128 * tc4:128 * (tc4 + 1)],
                                     wproj_sb[:, 1, TT * h:TT * (h + 1)],
                                     start=True, stop=True)
                warm()
                nc.scalar.copy(osb[:, 0:TT], pso2[:, 0:TT])
                nc.vector.tensor_copy(osb[:, TT:2 * TT], pso2[:, TT:2 * TT])
                for h in range(2):
                    ct = 2 + h
                    pso = unips.tile([128, TT], F32, tag="uni", name="psoBn")
                    nc.tensor.matmul(pso[:],
                                     ytj[:, 1, 128 * tc4:128 * (tc4 + 1)],
                                     wproj_sb[:, 1, TT * ct:TT * (ct + 1)],
                                     start=True, stop=True)
                    if h == 0:
                        nc.scalar.copy(osb[:, TT * ct:TT * (ct + 1)], pso[:])
                    else:
                        nc.vector.tensor_copy(osb[:, TT * ct:TT * (ct + 1)],
                                              pso[:])
                warm()
                eng = nc.sync if tc4 % 2 == 0 else nc.scalar
                eng.dma_start(
                    out[TT * 3 + 128 * tc4: TT * 3 + 128 * (tc4 + 1), :],
                    osb[:])

        # prefetch tile 2 as soon as tile-0's gemm is emitted (xtp/csp
        # have the slots); tile 3 follows during attention-0
        loads.append(load_tile(2))
        # wproj behind the tile-2 prefetch: first consumer is proj-0 at
        # ~55us, while the gemm-2 weave needs xh2 at ~46us
        nc.scalar.dma_start(wproj_sb[:], wproj_v[:])

        v3_stash = []
        for i in range(NTT):
            if i + 3 < NTT:
                loads.append(load_tile(i + 3))
            # build the weave filler: next tile's gemm (for the last tile,
            # which has no next gemm: its own deferred v-GEMM + previous
            # tile's deferred proj -- tile-3's attention is ACT(exp)-bound,
            # the mid tiles are PE-bound, so shift PE work rightward)
            if i + 1 < NTT:
                qkc, vc = gemm_chunks(i + 1)
                if i + 1 == NTT - 1:
                    chunks = qkc
                    v3_stash[:] = vc
                else:
                    chunks = qkc + vc
            elif i >= 1:
                chunks = v3_stash + proj_chunks(i - 1, yts[i - 1])
            else:
                chunks = []
            ci = 0  # chunk cursor

            qrot = qrots[i]
            yt = ytp.tile([128, 2, TT], BF16, tag="yt")
            yts[i] = yt
            nsb = 4 * (i + 1)
            nsteps = 2 * nsb
            step = 0
            for bp in range(2):
                psyA = yps.tile([65, TT], F32, tag="y")
                psyB = yps.tile([65, TT], F32, tag="y")

                def emit_scores(sb):
                    """scores pair + exp + corner masks for one s-block."""
                    dd = sb - 4 * i
                    toff = 128 * dd if dd >= 0 else 0
                    s2 = sps.tile([128, 2 * TT], F32, tag="S", name="s2")
                    nc.tensor.matmul(s2[:, toff:TT],
                                     krot[0:64, bp, 128 * sb:128 * (sb + 1)],
                                     qrot[0:64, bp, toff:TT],
                                     start=True, stop=True, tile_position=(0, 0))
                    nc.tensor.matmul(s2[:, TT:2 * TT - toff],
                                     krot[64:128, bp, 128 * sb:128 * (sb + 1)],
                                     qrot[64:128, bp, toff:TT],
                                     start=True, stop=True, tile_position=(64, 0))
                    p4 = p4p.tile([128, 2 * TT], BF16, tag="P4", name="p4")
                    nc.scalar.activation(p4[:, toff:2 * TT - toff],
                                         s2[:, toff:2 * TT - toff], EXP, scale=0.125)
                    if dd >= 0:
                        # only the 128-wide diagonal corner needs masking
                        nc.gpsimd.tensor_tensor(
                            p4[:, toff:toff + 128],
                            p4[:, toff:toff + 128], tri_sb[:], MUL)
                        nc.gpsimd.tensor_tensor(
                            p4[:, TT:TT + 128],
                            p4[:, TT:TT + 128], tri_sb[:], MUL)
                    return p4, toff

                def emit_av(sb, p4, toff):
                    nc.tensor.matmul(psyA[:, toff:TT], v_sb[:, sb, 2 * bp, 0:65],
                                     p4[:, toff:TT],
                                     start=(sb == 0), stop=(sb == nsb - 1))
                    nc.tensor.matmul(psyB[:, toff:TT], v_sb[:, sb, 2 * bp + 1, 0:65],
                                     p4[:, TT:2 * TT - toff],
                                     start=(sb == 0), stop=(sb == nsb - 1))

                # unrolled by 2: scores/exp run one s-block ahead of av so
                # the av LDWEIGHTS prefetch + exp latency hide behind the
                # next block's scores and the woven next-tile GEMM chunks
                for r in range(0, nsb, 2):
                    p4a, toffa = emit_scores(r)
                    p4b, toffb = emit_scores(r + 1)
                    step += 2
                    want = (ci if step <= nsteps // 4 else
                            ((len(chunks) * (step - nsteps // 4) * 4)
                             // (3 * nsteps) if nsteps >= 4 else len(chunks)))
                    if i == NTT - 1:
                        # faster ramp (tile-3 is exp-bound, PE has slack;
                        # an empty post-attention drain shortens the tail),
                        # and tile-3's own v chunks head its weave list:
                        # block 12+k's write must be EMITTED before its av
                        # reads it (Tile only orders reads after earlier
                        # writes)
                        want = max(want, (len(chunks) * step) // nsteps)
                        if r + 1 >= 12:
                            want = max(want, 4 * (r - 10))
                    while ci < min(want, len(chunks)):
                        chunks[ci]()
                        ci += 1
                    emit_av(r, p4a, toffa)
                    emit_av(r + 1, p4b, toffb)
                # per-bp tails, inline: the psum->SBUF evacuations ride the
                # DVE (ACT stays pure-exp for the next attention stretch),
                # then a couple of woven chunks cover the DVE latency before
                # the broadcast matmuls + yt multiplies.
                tails = []
                # junk keep-warm burst through the tile-3 tail: the tail's
                # dependency holes otherwise trip the HAM MID window and the
                # whole tail runs at 1.2 GHz
                if i == NTT - 1 and bp == 1:
                    wcell = {"n": 0}

                    def warm():
                        if wcell["n"] == 0:
                            wcell["ps"] = yps.tile([128, TT], F32, tag="y",
                                                   name="warmtail")
                        for _ in range(2):
                            nc.tensor.matmul(wcell["ps"][:, 0:256],
                                             warm_sb[:, 0:128], warm_sb[:],
                                             start=(wcell["n"] == 0), stop=False)
                            wcell["n"] += 1

                    def warm_close():
                        nc.tensor.matmul(wcell["ps"][:, 0:256],
                                         warm_sb[:, 0:128], warm_sb[:],
                                         start=False, stop=True)
                        nc.vector.tensor_copy(junk_sb[:], wcell["ps"][0:1, 0:8])
                else:
                    warm = lambda: None
                    warm_close = lambda: None
                for hh, psy in ((0, psyA), (1, psyB)):
                    ym65 = ymp.tile([65, TT], F32, tag="ym")
                    # split the psum evacuations across DVE and ACT so
                    # neither queue backs up at the bp boundary
                    if hh == 0:
                        nc.vector.tensor_copy(ym65[:], psy[:])
                    else:
                        nc.scalar.copy(ym65[:], psy[:])
                    # reciprocal over the WHOLE tile, partition-aligned (the
                    # custom DVE op mishandles shifted APs); rows 0-63 are
                    # unused garbage, row 64 is 1/denominator. Same per-lane
                    # cost as a 1-row op, and no staging copy needed.
                    rsb = rp.tile([65, TT], F32, tag="r")
                    nc.vector.reciprocal_approx_fast(out=rsb[:], in_=ym65[:])
                    # matmul rhs must be f32r-ROUNDED, not bitcast
                    rsbr = rp.tile([65, TT], F32R, tag="rr")
                    nc.vector.tensor_copy(rsbr[64:65, :], rsb[64:65, :])
                    tails.append((hh, ym65, rsbr))
                    if bp == 1:
                        warm()
                for _ in range(4 if i == NTT - 1 else 3):
                    if ci < len(chunks):
                        chunks[ci]()
                        ci += 1
                for hh, ym65, rsbr in tails:
                    psb = yps.tile([128, TT], F32, tag="y")
                    nc.tensor.matmul(psb[0:64, :], ones_sb[64:65, :],
                                     rsbr[64:65, :],
                                     start=True, stop=True, tile_position=(64, 0))
                    if bp == 1:
                        warm()
                    dst = yt[0:64, bp, :] if hh == 0 else yt[64:128, bp, :]
                    nc.vector.tensor_tensor(dst, ym65[0:64, :], psb[0:64, :], MUL)
                if i == NTT - 1 and bp == 0:
                    # yt[:, 0] is final now: tile-3's b=0 proj half joins
                    # the weave for the bp=1 attention stretch
                    chunks = chunks + passA_chunks(yt)

            # drain remaining weave work AFTER the yt chain is queued, so
            # the next consumer of yt (proj / passB) isn't stuck behind a
            # pile of drained evac-copies in the DVE FIFO
            while ci < len(chunks):
                chunks[ci]()
                ci += 1

            if i == NTT - 1:
                emit_passB(yt, warm)
                warm_close()
            elif i != NTT - 2:
                emit_proj(i, yt)

    nc.finalize()
    return nc


def _host_inputs(x, w_qkv, w_proj, attn_mask):
    """Build the 8 per-core input maps (host-side sharding/layout prep)."""
    import ml_dtypes

    BF = ml_dtypes.bfloat16
    x = np.asarray(x)
    w_qkv = np.asarray(w_qkv)
    w_proj = np.asarray(w_proj)
    attn_mask = np.asarray(attn_mask)

    xT = np.ascontiguousarray(x.reshape(T, C).T).astype(BF)

    # RoPE tables, faithful to the reference broadcasting quirk:
    # head g rotates all pairs by angle t * theta^(-g/32) (f32 math).
    inv_freq = (1.0 / (ROPE_THETA ** (np.arange(0, D, 2, dtype=np.float32) / D))
                ).astype(np.float32)                     # [32] indexed by head
    t_ar = np.arange(T, dtype=np.float32)
    freqs = (t_ar[:, None] * inv_freq[None, :]).astype(np.float32)  # [T, 32]
    cosf = np.cos(freqs).astype(np.float32)              # [T, 32]
    sinf = np.sin(freqs).astype(np.float32)
    # sigma folds the rotation sign into the PRE-swap sin scale:
    # dst[p] = q[p]*cos + q[p^1]*sgn[p]*sin with sgn[p] = -1 for even p.
    # qsin[q] = q[q]*sigma[q] must satisfy sigma[p^1] = sgn[p],
    # i.e. sigma[q] = sgn[q^1] = -sgn[q] = +1 for even q, -1 for odd q.
    sigma = np.where(np.arange(64) % 2 == 0, np.float32(1.0), np.float32(-1.0))

    # 0/1 keep-mask for the 128x128 diagonal corner, from the actual mask
    tri128 = np.exp(
        attn_mask[0:128, 0:128].astype(np.float64)).T.astype(BF)

    permM = np.zeros((128, 128), dtype=np.float32)
    permM[np.arange(128), np.arange(128) ^ 1] = 1.0
    permM = permM.astype(BF)

    in_maps = []
    for c in range(NC_):
        wqk_c = np.ascontiguousarray(np.concatenate(
            [w_qkv[:, 256 * c:256 * (c + 1)],
             w_qkv[:, 2048 + 256 * c:2048 + 256 * (c + 1)]], axis=1)).astype(BF)
        wv_c = np.ascontiguousarray(
            w_qkv[:, 4096 + 256 * c:4096 + 256 * (c + 1)]).astype(BF)
        wproj_c = np.ascontiguousarray(w_proj[256 * c:256 * (c + 1), :]).astype(BF)

        costab = np.empty((128, 2, T), dtype=np.float32)
        sintab = np.empty((128, 2, T), dtype=np.float32)
        for bb in range(2):
            for p in range(128):
                g = 4 * c + 2 * bb + (p // 64)           # global head
                costab[p, bb, :] = cosf[:, g]
                sintab[p, bb, :] = sigma[p % 64] * sinf[:, g]

        in_maps.append({
            "xt": xT, "wqk": wqk_c, "wv": wv_c, "wproj": wproj_c,
            "costab": costab.astype(BF), "sintab": sintab.astype(BF),
            "tri": tri128, "perm": permM,
        })
    return in_maps


def _get_program():
    if "nc" not in _CACHE:
        _CACHE["nc"] = _build_program()
    return _CACHE["nc"]


def run_sharded(in_maps, trace=False):
    from concourse.bass_utils import run_bass_kernel_spmd
    nc = _get_program()
    return run_bass_kernel_spmd(nc, in_maps, list(range(NC_)), trace=trace)


def gather(res):
    acc = res.results[0]["out"].astype(np.float32).copy()
    acc[TT * 3:] += res.results[0]["out2"]
    for c in range(1, NC_):
        acc += res.results[c]["out"]
        acc[TT * 3:] += res.results[c]["out2"]
    return acc.reshape(1, T, C)


def kernel(x, w_qkv, w_proj, attn_mask):
    in_maps = _host_inputs(x, w_qkv, w_proj, attn_mask)
    res = run_sharded(in_maps)
    return gather(res)



# revision 44
# speedup vs baseline: 1.0021x; 1.0021x over previous
"""Trainium2 Bass kernel for nn_MHA_2516850835986.

MHA: B=1, T=2048, C=2048, H=32 heads, d=64, causal, RoPE (head-indexed
angle quirk: within head h all feature pairs rotate by t * 10000^(-h/32)).

Sharding: head-parallel across 8 cores (4 heads each). x is replicated
(pre-transposed on host), qkv columns / proj rows sharded by head. Each
core produces a partial [T, C] output (proj contraction over its own
heads' features); partials are summed on host.

v6 design (v5 ~244us, v4 246.7us, v3 299us, f32r baseline 425us;
v6 best sample 236.8us, band ~237-246 dominated by cross-core HBM
phase variance):
- Preamble DMA in exact consumption order: wqk|xt quarters, then wv
  halves split across both queues (the v chains consume all 16 kc right
  after qk), then cos/sin. NWARM=24 bridges the warm-up to the first
  DMA-gated matmul so HAM opens at ~11us and stays open ~185us.
- passB rotates 4 psum chains (2-bank sps slab + two unips banks) so
  its evac copies pipeline fully behind the matmuls; junk keep-warm
  tile rides yps (free at the tail).

v5 design notes:
- ACT kept pure-exp through attention: all other psum evacuations ride
  DVE (GpSimd has no PSUM port); wide [128,1024] psum slabs evac'd with
  ACT+DVE halves in parallel during the exp-free proj windows.
- Per-bp inline softmax tails (no deferred pile-up at tile boundaries).
- Tile-3's v-GEMM + tile-2's proj + tile-3's b=0 proj half all weave
  into tile-3's exp-bound attention; the b=0 partial ships to a second
  DRAM output (out2) summed by the host like any core partial, so only
  the b=1 half + out-DMA remain after the last attention step.
- DMA: both HW queues carry the tile-0 critical stream in consumption
  order (wqk|xt quarters, then cos/sin, then wv split across queues);
  out-DMAs alternate queues; SWDGE only for the tiny masks (its early
  issue steals HBM bandwidth, and it is too slow for strided tables).
- Junk keep-warm matmuls thread the tile-3 tail so the HAM clock gate
  stays at 2.4 GHz through passB.

v4 design notes:
- bf16 matmul streams everywhere (tol 2e-2; measured v3 err 5.5e-3).
- Software-pipelined EMISSION: the PE executes in strict pc order, so
  tile i+1's qk/v GEMM is emitted in ~4-matmul chunks BETWEEN the
  scores->av steps of tile i's attention. The ~870ns exp latency per
  step is hidden behind next-tile GEMM work instead of stalling the PE.
- Tile-0 qk runs kc-outer (4 concurrent psum chains) so each arriving
  1MB DMA quarter immediately unlocks 16 matmuls: the preamble streams.
- Diagonal score blocks narrowed to causal width; only the 128-wide
  corner is masked (gpsimd, bf16).
- RoPE fused into the qk-psum evacuation (qcos/qsin), sign folded into
  sintab, swap via PE perm matmul.
- reciprocal_approx_fast on a partition-0 staged denominator row (the
  custom DVE op mishandles partition-shifted APs - learned the NaN way).
- PE warm-up burst at t=0 keeps the HAM clock gate at 2.4 GHz.
"""

import sys

sys.path.insert(0, "/opt/trn_rl_repo")
import numpy as np

T = 2048
C = 2048
NH = 32          # total heads
HL = 4           # heads per core
D = 64           # head dim
NC_ = 8          # cores
TT = 512         # t-tile width
NTT = T // TT    # 4 t-tiles
KC = C // 128    # 16 contraction chunks
ROPE_THETA = 10000.0

_CACHE = {}


def _build_program():
    import concourse.bass as bass
    import concourse.tile as tile
    from concourse import bacc, mybir
    from contextlib import ExitStack

    F32 = mybir.dt.float32
    F32R = mybir.dt.float32r
    BF16 = mybir.dt.bfloat16
    EXP = mybir.ActivationFunctionType.Exp
    MUL = mybir.AluOpType.mult
    ADD = mybir.AluOpType.add

    nc = bacc.Bacc(None, target_bir_lowering=False)

    xt = nc.declare_dram_parameter("xt", [C, T], BF16, False)          # x^T
    wqk = nc.declare_dram_parameter("wqk", [C, 4 * 128], BF16, False)  # q|k cols
    wv = nc.declare_dram_parameter("wv", [C, 256], BF16, False)
    wproj = nc.declare_dram_parameter("wproj", [256, T], BF16, False)
    costab = nc.declare_dram_parameter("costab", [128, 2, T], BF16, False)
    sintab = nc.declare_dram_parameter("sintab", [128, 2, T], BF16, False)
    tri = nc.declare_dram_parameter("tri", [128, 128], BF16, False)    # corner keep-mask
    perm = nc.declare_dram_parameter("perm", [128, 128], BF16, False)  # pair-swap
    out = nc.declare_dram_parameter("out", [T, T], BF16, True)
    # tile-3 proj b=0 partial (computed early, woven into tile-3's
    # attention; host adds it into rows 1536:2048 like any other partial)
    out2 = nc.declare_dram_parameter("out2", [TT, T], BF16, True)

    xt_v = xt.rearrange("(kc p) t -> p kc t", p=128)
    wqk_v = wqk.rearrange("(kc p) m -> p kc m", p=128)
    wv_v = wv.rearrange("(kc p) m -> p kc m", p=128)
    wproj_v = wproj.rearrange("(b p) n -> p b n", p=128)

    with tile.TileContext(nc) as tc, ExitStack() as ctx:
        consts = ctx.enter_context(tc.tile_pool(name="consts", bufs=1))
        xtp = ctx.enter_context(tc.tile_pool(name="xtp", bufs=6))
        csp = ctx.enter_context(tc.tile_pool(name="csp", bufs=4))
        ropep = ctx.enter_context(tc.tile_pool(name="ropep", bufs=2))
        qrotp = ctx.enter_context(tc.tile_pool(name="qrotp", bufs=2))
        persist = ctx.enter_context(tc.tile_pool(name="persist", bufs=1))
        p4p = ctx.enter_context(tc.tile_pool(name="p4p", bufs=2))
        ytp = ctx.enter_context(tc.tile_pool(name="ytp", bufs=2))
        ytmpp = ctx.enter_context(tc.tile_pool(name="ytmpp", bufs=2))
        ymp = ctx.enter_context(tc.tile_pool(name="ymp", bufs=4))
        rp = ctx.enter_context(tc.tile_pool(name="rp", bufs=4))
        outp = ctx.enter_context(tc.tile_pool(name="outp", bufs=4))

        # PSUM: S2 pairs (2 banks x2) + y (1 bank x2) + everything else (1 bank x2)
        sps = ctx.enter_context(tc.tile_pool(name="sps", bufs=2, space="PSUM"))
        yps = ctx.enter_context(tc.tile_pool(name="yps", bufs=2, space="PSUM"))
        unips = ctx.enter_context(tc.tile_pool(name="unips", bufs=2, space="PSUM"))

        wqk_sb = consts.tile([128, KC, 512], BF16)
        wv_sb = consts.tile([128, KC, 256], BF16)
        wproj_sb = consts.tile([128, 2, T], BF16)
        tri_sb = consts.tile([128, 128], BF16)
        perm_sb = consts.tile([128, 128], BF16)
        ones_sb = consts.tile([128, 64], F32R)
        nc.vector.memset(ones_sb[:].bitcast(F32), 1.0)

        # ---- PE warm-up: junk matmuls so the HAM activity window sees a
        # busy PE during the DMA preamble and the clock gate opens to
        # 2.4 GHz before the first real matmul ----
        warm_sb = consts.tile([128, 256], BF16)
        junk_sb = consts.tile([1, 8], F32)
        nc.gpsimd.memset(warm_sb[:], 0.25)
        wps = unips.tile([128, TT], F32, tag="uni")
        NWARM = 24
        for w in range(NWARM):
            nc.tensor.matmul(wps[:, 0:256], warm_sb[:, 0:128], warm_sb[:],
                             start=(w == 0), stop=(w == NWARM - 1))
        nc.vector.tensor_copy(junk_sb[:], wps[0:1, 0:8])  # keep-alive consumer

        # v in normal layout [s, dd]: per s-block slot of 4 heads x (64 v + 1 one + 1 pad)
        v_sb = persist.tile([128, KC, HL, 66], BF16)
        nc.vector.memset(v_sb[:].rearrange("p a b c -> p (a b c)"), 1.0)
        # k^T (rope'd), persistent across tiles: [dd(2 heads), block, t]
        krot = persist.tile([128, 2, T], BF16)

        def load_tile(j):
            """Issue input DMAs for t-tile j (xt halves split across the
            two HW queues, cos/sin right behind them)."""
            tslj = slice(TT * j, TT * (j + 1))
            xth = []
            for half in range(2):
                xh = xtp.tile([128, KC // 2, TT], BF16, tag="xt")
                eng = nc.sync if half == 0 else nc.scalar
                eng.dma_start(xh[:], xt_v[:, (KC // 2) * half:(KC // 2) * (half + 1), tslj])
                xth.append(xh)
            cos_t = csp.tile([128, 2, TT], BF16, tag="cos")
            nc.sync.dma_start(cos_t[:], costab[:, :, tslj])
            sin_t = csp.tile([128, 2, TT], BF16, tag="sin")
            nc.scalar.dma_start(sin_t[:], sintab[:, :, tslj])
            return xth, cos_t, sin_t

        # ---- preamble: tile-0 inputs interleaved with wqk in quarter
        # chunks across both HW queues, in exact consumption order; only
        # the tiny masks ride the gpsimd SWDGE queue (big transfers there
        # would steal HBM bandwidth from the critical stream since SWDGE
        # issues immediately) ----
        xh0 = xtp.tile([128, KC // 2, TT], BF16, tag="xt")
        xh1 = xtp.tile([128, KC // 2, TT], BF16, tag="xt")
        xq = [xh0[:, 0:4, :], xh0[:, 4:8, :], xh1[:, 0:4, :], xh1[:, 4:8, :]]
        nc.gpsimd.dma_start(perm_sb[:], perm[:])
        nc.gpsimd.dma_start(tri_sb[:], tri[:])
        for q in range(4):
            nc.sync.dma_start(wqk_sb[:, 4 * q:4 * (q + 1), :],
                              wqk_v[:, 4 * q:4 * (q + 1), :])
            nc.scalar.dma_start(xq[q], xt_v[:, 4 * q:4 * (q + 1), 0:TT])
        # wv halves FIRST (the v chains consume all 16 kc chunks right
        # after the qk chains, ~2us before the rope needs cos/sin)
        nc.sync.dma_start(wv_sb[:, 0:8, :], wv_v[:, 0:8, :])
        nc.scalar.dma_start(wv_sb[:, 8:16, :], wv_v[:, 8:16, :])
        cos0 = csp.tile([128, 2, TT], BF16, tag="cos")
        nc.sync.dma_start(cos0[:], costab[:, :, 0:TT])
        sin0 = csp.tile([128, 2, TT], BF16, tag="sin")
        nc.scalar.dma_start(sin0[:], sintab[:, :, 0:TT])
        loads = [([xh0, xh1], cos0, sin0)]
        # prefetch tile 1 behind the tile-0 critical stream; wproj last
        # (first consumer is tile-0's proj at ~60us)
        loads.append(load_tile(1))
        nc.sync.dma_start(wproj_sb[:, 0:1, :], wproj_v[:, 0:1, :])
        nc.scalar.dma_start(wproj_sb[:, 1:2, :], wproj_v[:, 1:2, :])

        qrots = {}
        yts = {}

        def emit_rope(m, ps, cos_t, sin_t, qrot, i):
            """Fused RoPE evacuation of one qk psum chain."""
            bb = m % 2
            qcos = ropep.tile([128, TT], BF16, tag="qcos")
            nc.vector.tensor_tensor(qcos[:], ps[:], cos_t[:, bb, :], MUL)
            qsin = ropep.tile([128, TT], BF16, tag="qsin")
            nc.vector.tensor_tensor(qsin[:], ps[:], sin_t[:, bb, :], MUL)
            psw = unips.tile([128, TT], F32, tag="uni")
            nc.tensor.matmul(psw[:], perm_sb[:], qsin[:], start=True, stop=True)
            dst = qrot[:, bb, :] if m < 2 else krot[:, bb, TT * i:TT * (i + 1)]
            nc.vector.tensor_tensor(dst, qcos[:], psw[:], ADD)

        def gemm_chunks(i):
            """Build tile i's qk+v GEMM as two lists of closures (qk+rope,
            then v), each chunk emitting ~4 matmuls, to be woven between
            attention steps."""
            xth, cos_t, sin_t = loads[i]
            qrot = qrotp.tile([128, 2, TT], BF16, tag="qrot")
            qrots[i] = qrot
            chunks = []
            for m in range(4):
                cell = {}

                def qk_chunk(m=m, q4=0, cell=cell):
                    if q4 == 0:
                        cell["ps"] = unips.tile([128, TT], F32, tag="uni", name="ps")
                    ps = cell["ps"]
                    for kc in range(4 * q4, 4 * q4 + 4):
                        nc.tensor.matmul(ps[:], wqk_sb[:, kc, 128 * m:128 * (m + 1)],
                                         xth[kc // 8][:, kc % 8, :],
                                         start=(kc == 0), stop=(kc == KC - 1))
                    if q4 == 3:
                        emit_rope(m, ps, cos_t, sin_t, qrot, i)

                for q4 in range(4):
                    chunks.append(lambda m=m, q4=q4, cell=cell: qk_chunk(m, q4, cell))
            vchunks = []
            for tc4 in range(4):
                cell = {}

                def v_chunk(tc4=tc4, q4=0, cell=cell):
                    if q4 == 0:
                        cell["ps"] = unips.tile([128, TT], F32, tag="uni", name="psv")
                    psv = cell["ps"]
                    for kc in range(4 * q4, 4 * q4 + 4):
                        nc.tensor.matmul(psv[:, 0:256],
                                         xth[kc // 8][:, kc % 8, 128 * tc4:128 * (tc4 + 1)],
                                         wv_sb[:, kc, :],
                                         start=(kc == 0), stop=(kc == KC - 1))
                    if q4 == 3:
                        nc.vector.tensor_copy(
                            v_sb[:, 4 * i + tc4, :, 0:64],
                            psv[:, 0:256].rearrange("p (h d) -> p h d", h=HL))

                for q4 in range(4):
                    vchunks.append(lambda tc4=tc4, q4=q4, cell=cell: v_chunk(tc4, q4, cell))
            return chunks, vchunks

        # ---- tile 0 GEMM inline, kc-outer so each arriving DMA quarter
        # (wqk q + xt q) unlocks 16 matmuls across 4 concurrent chains ----
        xth0, cos_t0, sin_t0 = loads[0]
        qrot0 = qrotp.tile([128, 2, TT], BF16, tag="qrot")
        qrots[0] = qrot0
        ps_m = [unips.tile([128, TT], F32, tag="uni", name="ps_m0"),
                unips.tile([128, TT], F32, tag="uni", name="ps_m1"),
                yps.tile([128, TT], F32, tag="y", name="ps_m2"),
                yps.tile([128, TT], F32, tag="y", name="ps_m3")]
        jps = sps.tile([128, 2 * TT], F32, tag="S", name="jps")
        nj = 0
        for kc in range(KC):
            for m in range(4):
                nc.tensor.matmul(ps_m[m][:], wqk_sb[:, kc, 128 * m:128 * (m + 1)],
                                 xth0[kc // 8][:, kc % 8, :],
                                 start=(kc == 0), stop=(kc == KC - 1))
            if kc % 4 == 3 and kc < KC - 1:
                for w in range(10):
                    nc.tensor.matmul(jps[:, 0:256], warm_sb[:, 0:128], warm_sb[:],
                                     start=(nj == 0), stop=(nj == 29))
                    nj += 1
        nc.vector.tensor_copy(junk_sb[:], jps[0:1, 0:8])  # release the S slot
        # v chain first, then rope: the PE streams the v GEMM while the DVE
        # works through the rope's elementwise ops
        for m in range(4):
            psv = unips.tile([128, TT], F32, tag="uni", name="psv0")
            for kc in range(KC):
                nc.tensor.matmul(psv[:, 0:256],
                                 xth0[kc // 8][:, kc % 8, 128 * m:128 * (m + 1)],
                                 wv_sb[:, kc, :],
                                 start=(kc == 0), stop=(kc == KC - 1))
            emit_rope(m, ps_m[m], cos_t0, sin_t0, qrot0, 0)
            nc.vector.tensor_copy(
                v_sb[:, m, :, 0:64],
                psv[:, 0:256].rearrange("p (h d) -> p h d", h=HL))

        def proj_block(j, ytj, tc4, ct, cell, pool, dve_only=False):
            """One [128,512] slab of tile j's proj: 2 matmuls + copy (+DMA)."""
            if ct == 0:
                cell["osb"] = outp.tile([128, 4 * TT], BF16, tag="osb", name="osb")
            osb = cell["osb"]
            pso = pool.tile([128, TT], F32, tag=("uni" if pool is unips else "y"),
                            name="pso")
            for b in range(2):
                nc.tensor.matmul(pso[:],
                                 ytj[:, b, 128 * tc4:128 * (tc4 + 1)],
                                 wproj_sb[:, b, TT * ct:TT * (ct + 1)],
                                 start=(b == 0), stop=(b == 1))
            if ct % 2 == 0 and not dve_only:
                nc.scalar.copy(osb[:, TT * ct:TT * (ct + 1)], pso[:])
            else:
                nc.vector.tensor_copy(osb[:, TT * ct:TT * (ct + 1)], pso[:])
            if ct == 3:
                nc.sync.dma_start(
                    out[TT * j + 128 * tc4: TT * j + 128 * (tc4 + 1), :],
                    osb[:])

        def emit_proj(j, ytj):
            """Partial out rows for t-tile j: [128,1024] psum slabs (wide
            bf16 moving operand), one wide evac copy per slab, out-DMAs
            balanced across both HW queues."""
            for tc4 in range(4):
                osb = outp.tile([128, 4 * TT], BF16, tag="osb", name="osb")
                for cp in range(2):
                    pso2 = sps.tile([128, 2 * TT], F32, tag="S", name="pso2")
                    for h in range(2):
                        ct = 2 * cp + h
                        for b in range(2):
                            nc.tensor.matmul(pso2[:, TT * h:TT * (h + 1)],
                                             ytj[:, b, 128 * tc4:128 * (tc4 + 1)],
                                             wproj_sb[:, b, TT * ct:TT * (ct + 1)],
                                             start=(b == 0), stop=(b == 1))
                    # split the wide evac across ACT+DVE so they run
                    # concurrently (no exp in this window)
                    nc.scalar.copy(osb[:, 2 * TT * cp:2 * TT * cp + TT],
                                   pso2[:, 0:TT])
                    nc.vector.tensor_copy(osb[:, 2 * TT * cp + TT:2 * TT * (cp + 1)],
                                          pso2[:, TT:2 * TT])
                eng = nc.sync if tc4 % 2 == 0 else nc.scalar
                eng.dma_start(
                    out[TT * j + 128 * tc4: TT * j + 128 * (tc4 + 1), :],
                    osb[:])

        def proj_chunks(j, ytj):
            """Tile j's proj as weave chunks (pso from the then-idle unips
            pool; copies DVE-only so the weave never steals ACT from the
            host attention's exp stream)."""
            chunks = []
            for tc4 in range(4):
                cell = {}
                for ct in range(4):
                    chunks.append(
                        lambda tc4=tc4, ct=ct, cell=cell:
                            proj_block(j, ytj, tc4, ct, cell, unips, True))
            return chunks

        def passA_chunks(ytj):
            """Tile-3 proj, b=0 contraction half only, as weave chunks.
            Each slab is 1 matmul + a DVE copy into an out2 staging tile;
            the host sums out2 into the final rows like any core partial."""
            chunks = []
            for tc4 in range(4):
                cell = {}

                def f(tc4=tc4, ct=0, cell=cell):
                    if ct == 0:
                        cell["osb"] = outp.tile([128, 4 * TT], BF16, tag="osb",
                                                name="osbA")
                    osb = cell["osb"]
                    pso = unips.tile([128, TT], F32, tag="uni", name="psoA")
                    nc.tensor.matmul(pso[:],
                                     ytj[:, 0, 128 * tc4:128 * (tc4 + 1)],
                                     wproj_sb[:, 0, TT * ct:TT * (ct + 1)],
                                     start=True, stop=True)
                    # mostly DVE; one in four on ACT (its exp stream has a
                    # little slack in the tile-3 bp=1 window, DVE has less)
                    if ct == 1:
                        nc.scalar.copy(osb[:, TT * ct:TT * (ct + 1)], pso[:])
                    else:
                        nc.vector.tensor_copy(osb[:, TT * ct:TT * (ct + 1)], pso[:])
                    if ct == 3:
                        nc.scalar.dma_start(
                            out2[128 * tc4:128 * (tc4 + 1), :], osb[:])

                for ct in range(4):
                    chunks.append(lambda tc4=tc4, ct=ct, cell=cell: f(tc4, ct, cell))
            return chunks

        def emit_passB(ytj, warm):
            """Tile-3 proj b=1 half: paired [128,1024] psum slabs from the
            (now idle) sps pool, evacs split across ACT+DVE, junk matmuls
            interleaved to hold the HAM clock gate open through the tail."""
            for tc4 in range(4):
                osb = outp.tile([128, 4 * TT], BF16, tag="osb", name="osbB")
                # cp=0 slab on a wide sps tile, cp=1 as two narrow unips
                # tiles: 4 psum chains rotate so the evac copies pipeline
                # fully behind the matmuls instead of gating them
                pso2 = sps.tile([128, 2 * TT], F32, tag="S", name="psoB")
                for h in range(2):
                    nc.tensor.matmul(pso2[:, TT * h:TT * (h + 1)],
                                     ytj[:, 1, 128 * tc4:128 * (tc4 + 1)],
                                     wproj_sb[:, 1, TT * h:TT * (h + 1)],
                                     start=True, stop=True)
                warm()
                nc.scalar.copy(osb[:, 0:TT], pso2[:, 0:TT])
                nc.vector.tensor_copy(osb[:, TT:2 * TT], pso2[:, TT:2 * TT])
                # ship each half-row as soon as its copies land: the final
                # out-DMA flush is the tail's critical path, so start it
                # ~2us earlier and halve the last transfer
                eng = nc.sync if tc4 % 2 == 0 else nc.scalar
                eng.dma_start(
                    out[TT * 3 + 128 * tc4: TT * 3 + 128 * (tc4 + 1), 0:2 * TT],
                    osb[:, 0:2 * TT])
                for h in range(2):
                    ct = 2 + h
                    pso = unips.tile([128, TT], F32, tag="uni", name="psoBn")
                    nc.tensor.matmul(pso[:],
                                     ytj[:, 1, 128 * tc4:128 * (tc4 + 1)],
                                     wproj_sb[:, 1, TT * ct:TT * (ct + 1)],
                                     start=True, stop=True)
                    if h == 0:
                        nc.scalar.copy(osb[:, TT * ct:TT * (ct + 1)], pso[:])
                    else:
                        nc.vector.tensor_copy(osb[:, TT * ct:TT * (ct + 1)],
                                              pso[:])
                warm()
                eng2 = nc.scalar if tc4 % 2 == 0 else nc.sync
                eng2.dma_start(
                    out[TT * 3 + 128 * tc4: TT * 3 + 128 * (tc4 + 1), 2 * TT:4 * TT],
                    osb[:, 2 * TT:4 * TT])

        # prefetch tile 2 as soon as tile-0's gemm is emitted (xtp/csp
        # have the slots); tile 3 follows during attention-0
        loads.append(load_tile(2))

        v3_stash = []
        for i in range(NTT):
            if i + 3 < NTT:
                loads.append(load_tile(i + 3))
            # build the weave filler: next tile's gemm (for the last tile,
            # which has no next gemm: its own deferred v-GEMM + previous
            # tile's deferred proj -- tile-3's attention is ACT(exp)-bound,
            # the mid tiles are PE-bound, so shift PE work rightward)
            if i + 1 < NTT:
                qkc, vc = gemm_chunks(i + 1)
                if i + 1 == NTT - 1:
                    chunks = qkc
                    v3_stash[:] = vc
                else:
                    chunks = qkc + vc
            elif i >= 1:
                chunks = v3_stash + proj_chunks(i - 1, yts[i - 1])
            else:
                chunks = []
            ci = 0  # chunk cursor

            qrot = qrots[i]
            yt = ytp.tile([128, 2, TT], BF16, tag="yt")
            yts[i] = yt
            nsb = 4 * (i + 1)
            nsteps = 2 * nsb
            step = 0
            for bp in range(2):
                psyA = yps.tile([65, TT], F32, tag="y")
                psyB = yps.tile([65, TT], F32, tag="y")

                def emit_scores(sb):
                    """scores pair + exp + corner masks for one s-block."""
                    dd = sb - 4 * i
                    toff = 128 * dd if dd >= 0 else 0
                    s2 = sps.tile([128, 2 * TT], F32, tag="S", name="s2")
                    nc.tensor.matmul(s2[:, toff:TT],
                                     krot[0:64, bp, 128 * sb:128 * (sb + 1)],
                                     qrot[0:64, bp, toff:TT],
                                     start=True, stop=True, tile_position=(0, 0))
                    nc.tensor.matmul(s2[:, TT:2 * TT - toff],
                                     krot[64:128, bp, 128 * sb:128 * (sb + 1)],
                                     qrot[64:128, bp, toff:TT],
                                     start=True, stop=True, tile_position=(64, 0))
                    p4 = p4p.tile([128, 2 * TT], BF16, tag="P4", name="p4")
                    nc.scalar.activation(p4[:, toff:2 * TT - toff],
                                         s2[:, toff:2 * TT - toff], EXP, scale=0.125)
                    if dd >= 0:
                        # only the 128-wide diagonal corner needs masking
                        nc.gpsimd.tensor_tensor(
                            p4[:, toff:toff + 128],
                            p4[:, toff:toff + 128], tri_sb[:], MUL)
                        nc.gpsimd.tensor_tensor(
                            p4[:, TT:TT + 128],
                            p4[:, TT:TT + 128], tri_sb[:], MUL)
                    return p4, toff

                def emit_av(sb, p4, toff):
                    nc.tensor.matmul(psyA[:, toff:TT], v_sb[:, sb, 2 * bp, 0:65],
                                     p4[:, toff:TT],
                                     start=(sb == 0), stop=(sb == nsb - 1))
                    nc.tensor.matmul(psyB[:, toff:TT], v_sb[:, sb, 2 * bp + 1, 0:65],
                                     p4[:, TT:2 * TT - toff],
                                     start=(sb == 0), stop=(sb == nsb - 1))

                # unrolled by 2: scores/exp run one s-block ahead of av so
                # the av LDWEIGHTS prefetch + exp latency hide behind the
                # next block's scores and the woven next-tile GEMM chunks
                for r in range(0, nsb, 2):
                    p4a, toffa = emit_scores(r)
                    p4b, toffb = emit_scores(r + 1)
                    step += 2
                    want = (ci if step <= nsteps // 4 else
                            ((len(chunks) * (step - nsteps // 4) * 4)
                             // (3 * nsteps) if nsteps >= 4 else len(chunks)))
                    if i == NTT - 1:
                        # faster ramp (tile-3 is exp-bound, PE has slack;
                        # an empty post-attention drain shortens the tail),
                        # and tile-3's own v chunks head its weave list:
                        # block 12+k's write must be EMITTED before its av
                        # reads it (Tile only orders reads after earlier
                        # writes)
                        want = max(want, (len(chunks) * step) // nsteps)
                        if r + 1 >= 12:
                            want = max(want, 4 * (r - 10))
                    while ci < min(want, len(chunks)):
                        chunks[ci]()
                        ci += 1
                    emit_av(r, p4a, toffa)
                    emit_av(r + 1, p4b, toffb)
                # per-bp tails, inline: the psum->SBUF evacuations ride the
                # DVE (ACT stays pure-exp for the next attention stretch),
                # then a couple of woven chunks cover the DVE latency before
                # the broadcast matmuls + yt multiplies.
                tails = []
                # junk keep-warm burst through the tile-3 tail: the tail's
                # dependency holes otherwise trip the HAM MID window and the
                # whole tail runs at 1.2 GHz
                if i == NTT - 1 and bp == 1:
                    wcell = {"n": 0}

                    def warm():
                        if wcell["n"] == 0:
                            wcell["ps"] = yps.tile([128, TT], F32, tag="y",
                                                   name="warmtail")
                        for _ in range(2):
                            nc.tensor.matmul(wcell["ps"][:, 0:256],
                                             warm_sb[:, 0:128], warm_sb[:],
                                             start=(wcell["n"] == 0), stop=False)
                            wcell["n"] += 1

                    def warm_close():
                        nc.tensor.matmul(wcell["ps"][:, 0:256],
                                         warm_sb[:, 0:128], warm_sb[:],
                                         start=False, stop=True)
                        nc.vector.tensor_copy(junk_sb[:], wcell["ps"][0:1, 0:8])
                else:
                    warm = lambda: None
                    warm_close = lambda: None
                for hh, psy in ((0, psyA), (1, psyB)):
                    ym65 = ymp.tile([65, TT], F32, tag="ym")
                    # split the psum evacuations across DVE and ACT so
                    # neither queue backs up at the bp boundary
                    if hh == 0:
                        nc.vector.tensor_copy(ym65[:], psy[:])
                    else:
                        nc.scalar.copy(ym65[:], psy[:])
                    # reciprocal over the WHOLE tile, partition-aligned (the
                    # custom DVE op mishandles shifted APs); rows 0-63 are
                    # unused garbage, row 64 is 1/denominator. Same per-lane
                    # cost as a 1-row op, and no staging copy needed.
                    rsb = rp.tile([65, TT], F32, tag="r")
                    nc.vector.reciprocal_approx_fast(out=rsb[:], in_=ym65[:])
                    # matmul rhs must be f32r-ROUNDED, not bitcast
                    rsbr = rp.tile([65, TT], F32R, tag="rr")
                    nc.vector.tensor_copy(rsbr[64:65, :], rsb[64:65, :])
                    tails.append((hh, ym65, rsbr))
                    if bp == 1:
                        warm()
                for _ in range(4 if i == NTT - 1 else 3):
                    if ci < len(chunks):
                        chunks[ci]()
                        ci += 1
                for hh, ym65, rsbr in tails:
                    psb = yps.tile([128, TT], F32, tag="y")
                    nc.tensor.matmul(psb[0:64, :], ones_sb[64:65, :],
                                     rsbr[64:65, :],
                                     start=True, stop=True, tile_position=(64, 0))
                    if bp == 1:
                        warm()
                    dst = yt[0:64, bp, :] if hh == 0 else yt[64:128, bp, :]
                    nc.vector.tensor_tensor(dst, ym65[0:64, :], psb[0:64, :], MUL)
                if i == NTT - 1 and bp == 0:
                    # yt[:, 0] is final now: tile-3's b=0 proj half joins
                    # the weave for the bp=1 attention stretch
                    chunks = chunks + passA_chunks(yt)

            # drain remaining weave work AFTER the yt chain is queued, so
            # the next consumer of yt (proj / passB) isn't stuck behind a
            # pile of drained evac-copies in the DVE FIFO
            while ci < len(chunks):
                chunks[ci]()
                ci += 1

            if i == NTT - 1:
                emit_passB(yt, warm)
                warm_close()
            elif i != NTT - 2:
                emit_proj(i, yt)

    nc.finalize()
    return nc


def _host_inputs(x, w_qkv, w_proj, attn_mask):
    """Build the 8 per-core input maps (host-side sharding/layout prep)."""
    import ml_dtypes

    BF = ml_dtypes.bfloat16
    x = np.asarray(x)
    w_qkv = np.asarray(w_qkv)
    w_proj = np.asarray(w_proj)
    attn_mask = np.asarray(attn_mask)

    xT = np.ascontiguousarray(x.reshape(T, C).T).astype(BF)

    # RoPE tables, faithful to the reference broadcasting quirk:
    # head g rotates all pairs by angle t * theta^(-g/32) (f32 math).
    inv_freq = (1.0 / (ROPE_THETA ** (np.arange(0, D, 2, dtype=np.float32) / D))
                ).astype(np.float32)                     # [32] indexed by head
    t_ar = np.arange(T, dtype=np.float32)
    freqs = (t_ar[:, None] * inv_freq[None, :]).astype(np.float32)  # [T, 32]
    cosf = np.cos(freqs).astype(np.float32)              # [T, 32]
    sinf = np.sin(freqs).astype(np.float32)
    # sigma folds the rotation sign into the PRE-swap sin scale:
    # dst[p] = q[p]*cos + q[p^1]*sgn[p]*sin with sgn[p] = -1 for even p.
    # qsin[q] = q[q]*sigma[q] must satisfy sigma[p^1] = sgn[p],
    # i.e. sigma[q] = sgn[q^1] = -sgn[q] = +1 for even q, -1 for odd q.
    sigma = np.where(np.arange(64) % 2 == 0, np.float32(1.0), np.float32(-1.0))

    # 0/1 keep-mask for the 128x128 diagonal corner, from the actual mask
    tri128 = np.exp(
        attn_mask[0:128, 0:128].astype(np.float64)).T.astype(BF)

    permM = np.zeros((128, 128), dtype=np.float32)
    permM[np.arange(128), np.arange(128) ^ 1] = 1.0
    permM = permM.astype(BF)

    in_maps = []
    for c in range(NC_):
        wqk_c = np.ascontiguousarray(np.concatenate(
            [w_qkv[:, 256 * c:256 * (c + 1)],
             w_qkv[:, 2048 + 256 * c:2048 + 256 * (c + 1)]], axis=1)).astype(BF)
        wv_c = np.ascontiguousarray(
            w_qkv[:, 4096 + 256 * c:4096 + 256 * (c + 1)]).astype(BF)
        wproj_c = np.ascontiguousarray(w_proj[256 * c:256 * (c + 1), :]).astype(BF)

        costab = np.empty((128, 2, T), dtype=np.float32)
        sintab = np.empty((128, 2, T), dtype=np.float32)
        for bb in range(2):
            for p in range(128):
                g = 4 * c + 2 * bb + (p // 64)           # global head
                costab[p, bb, :] = cosf[:, g]
                sintab[p, bb, :] = sigma[p % 64] * sinf[:, g]

        in_maps.append({
            "xt": xT, "wqk": wqk_c, "wv": wv_c, "wproj": wproj_c,
            "costab": costab.astype(BF), "sintab": sintab.astype(BF),
            "tri": tri128, "perm": permM,
        })
    return in_maps


def _get_program():
    if "nc" not in _CACHE:
        _CACHE["nc"] = _build_program()
    return _CACHE["nc"]


def run_sharded(in_maps, trace=False):
    from concourse.bass_utils import run_bass_kernel_spmd
    nc = _get_program()
    return run_bass_kernel_spmd(nc, in_maps, list(range(NC_)), trace=trace)


def gather(res):
    acc = res.results[0]["out"].astype(np.float32).copy()
    acc[TT * 3:] += res.results[0]["out2"]
    for c in range(1, NC_):
        acc += res.results[c]["out"]
        acc[TT * 3:] += res.results[c]["out2"]
    return acc.reshape(1, T, C)


def kernel(x, w_qkv, w_proj, attn_mask):
    in_maps = _host_inputs(x, w_qkv, w_proj, attn_mask)
    res = run_sharded(in_maps)
    return gather(res)

